# revision 1
# baseline (speedup 1.0000x reference)
"""Fused SwiGLU MLP (gate/up/down) Trainium2 Bass kernel.

Problem: y = down( silu(x @ Wg^T) * (x @ Wu^T) ) with
  x  [B=2, S=2048, H=4096]  f32
  Wg [I=11008, H]           f32   (gate proj, [out,in])
  Wu [I=11008, H]           f32
  Wd [H, I]                 f32

Strategy: data-parallel over tokens across the 8 NeuronCores.
Each core gets T = 4096/8 = 512 tokens and the full (replicated) weights,
computing the entire MLP for its token shard.  No collectives; the host
just concatenates the 8 token shards.  Per-core work: 138.6 GFLOP
(compute-bound: ~1.8 ms at the 78.6 TFLOP/s f32 PE roofline) vs ~532 MiB
of HBM traffic (~1.5 ms at ~360 GB/s), overlapped.

Device-side layout decisions (all transposes/tiling done on HOST in numpy
so every device DMA is a plain contiguous partition-major copy):
  x_host  [128, HS=32, T]          x^T tiled: [p, hs, t] = x[t, hs*128+p]
  wg_host [IC=22, 16, 128, 2, 512] Wg^T tiled (I padded 11008->11264)
  wu_host same
  wd_host [IC, 8, 128, 4, 512]     Wd^T tiled
  y out   [TT=4, 128, H]           y[tt*128+p, o]  (natural token-major)

Per-core kernel (per i-chunk ic of 512 padded-I columns):
  gate/up:  psum_g/u[it][128i, T] += Wg^T[h,i].T @ x^T[h,t]   (32 h-subtiles)
  mid:      hm[it] = silu(psum_g) * psum_u          (ACT + DVE)
  down:     psum_y[128t, 512o]    += hm[is][:,tt].T-as-lhsT @ Wd^T[i,o]
            y_sbuf[tt] += psum_y                    (DVE accumulate)
"""

import numpy as np

import concourse.bass as bass
import concourse.mybir as mybir
import concourse.tile as tile
from concourse import bacc
from concourse.bass_utils import run_bass_kernel_spmd

F32 = mybir.dt.float32
F32R = mybir.dt.float32r
P = 128
ICW = 512  # i-chunk width (4 subtiles of 128)
OCW = 512  # o-chunk width

# full-size problem constants
B, S, H, I = 2, 2048, 4096, 11008
NCORES = 8
T = (B * S) // NCORES  # 512 tokens per core
IPAD = 11264           # 22 * 512


def build_nc(T, H, IPAD, wg_bufs=6, wd_bufs=4, hm_bufs=5, sg_bufs=2, mm_dt=F32,
             use_silu=True):
    assert T % P == 0 and T <= 512
    assert H % 512 == 0 and (H // P) % 2 == 0
    assert IPAD % ICW == 0
    HS = H // P       # h subtiles (contraction for gate/up)
    IC = IPAD // ICW  # i chunks
    NO = H // OCW     # o chunks
    TT = T // P       # token tiles

    nc = bacc.Bacc("TRN2", target_bir_lowering=False, debug=False)
    x_d = nc.dram_tensor("x", [P, HS, T], mm_dt, kind="ExternalInput").ap()
    wg_d = nc.dram_tensor("wg", [IC, HS // 2, P, 2, ICW], mm_dt, kind="ExternalInput").ap()
    wu_d = nc.dram_tensor("wu", [IC, HS // 2, P, 2, ICW], mm_dt, kind="ExternalInput").ap()
    wd_d = nc.dram_tensor("wd", [IC, NO, P, ICW // P, OCW], mm_dt, kind="ExternalInput").ap()
    y_d = nc.dram_tensor("y", [TT, P, H], F32, kind="ExternalOutput").ap()

    with tile.TileContext(nc) as tc:
        with (
            tc.tile_pool(name="xp", bufs=1) as xp,
            tc.tile_pool(name="yp", bufs=1) as yp,
            tc.tile_pool(name="wgp", bufs=wg_bufs) as wgp,
            tc.tile_pool(name="wup", bufs=wg_bufs) as wup,
            tc.tile_pool(name="wdp", bufs=wd_bufs) as wdp,
            tc.tile_pool(name="hmp", bufs=hm_bufs) as hmp,
            tc.tile_pool(name="sgp", bufs=sg_bufs) as sgp,
            tc.tile_pool(name="ps", bufs=8, space="PSUM") as ps,
        ):
            # resident x^T (8 MiB) and y accumulator (8 MiB)
            xt = xp.tile([P, HS, T], mm_dt)
            nc.sync.dma_start(out=xt, in_=x_d)
            yt = []
            for tt in range(TT):
                ytile = yp.tile([P, H], F32, name=f"y{tt}", tag=f"y{tt}")
                nc.vector.memset(ytile, 0.0)
                yt.append(ytile)

            for ic in range(IC):
                # ---- gate/up projections, accumulated over all h ----
                psg = [ps.tile([P, T], F32, tag="ps", name=f"psg{k}") for k in range(4)]
                psu = [ps.tile([P, T], F32, tag="ps", name=f"psu{k}") for k in range(4)]
                for j in range(HS // 2):
                    gt = wgp.tile([P, 2, ICW], mm_dt, tag="wg")
                    nc.sync.dma_start(out=gt, in_=wg_d[ic, j])
                    ut = wup.tile([P, 2, ICW], mm_dt, tag="wu")
                    nc.sync.dma_start(out=ut, in_=wu_d[ic, j])
                    for h2 in range(2):
                        hs = 2 * j + h2
                        first, last = hs == 0, hs == HS - 1
                        for it in range(4):
                            nc.tensor.matmul(
                                psg[it],
                                gt[:, h2, it * P:(it + 1) * P],
                                xt[:, hs, :],
                                start=first, stop=last,
                            )
                        for it in range(4):
                            nc.tensor.matmul(
                                psu[it],
                                ut[:, h2, it * P:(it + 1) * P],
                                xt[:, hs, :],
                                start=first, stop=last,
                            )
                # ---- silu(gate) * up -> hm tiles [i128, T] ----
                hms = []
                for it in range(4):
                    sg = sgp.tile([P, T], F32, tag="sg")
                    if use_silu:
                        # native HW silu: one ACT op frees psg immediately
                        nc.scalar.activation(
                            sg, psg[it], mybir.ActivationFunctionType.Silu
                        )
                    else:
                        # CoreSim lacks Silu: sigmoid + extra DVE mul
                        nc.scalar.activation(
                            sg, psg[it], mybir.ActivationFunctionType.Sigmoid
                        )
                        nc.vector.tensor_mul(sg, sg, psg[it])
                    hm = hmp.tile([P, T], mm_dt, tag="hm")
                    nc.vector.tensor_mul(hm, sg, psu[it])
                    hms.append(hm)
                # ---- down projection for this i-chunk ----
                ISUB = ICW // P
                for osc in range(NO):
                    # wd for this (ic, osc) in two half tiles to keep SBUF slim
                    wdts = []
                    for half in range(2):
                        wdt = wdp.tile([P, ISUB // 2, OCW], mm_dt, tag="wd", name=f"wd{half}")
                        nc.sync.dma_start(
                            out=wdt,
                            in_=wd_d[ic, osc, :, half * (ISUB // 2):(half + 1) * (ISUB // 2), :],
                        )
                        wdts.append(wdt)
                    for tt in range(TT):
                        py = ps.tile([P, OCW], F32, tag="ps", name="py")
                        for isub in range(ISUB):
                            nc.tensor.matmul(
                                py,
                                hms[isub][:, tt * P:(tt + 1) * P],
                                wdts[isub // (ISUB // 2)][:, isub % (ISUB // 2), :],
                                start=(isub == 0), stop=(isub == ISUB - 1),
                            )
                        osl = slice(osc * OCW, (osc + 1) * OCW)
                        nc.vector.tensor_add(yt[tt][:, osl], yt[tt][:, osl], py)

            for tt in range(TT):
                nc.sync.dma_start(out=y_d[tt], in_=yt[tt])

    nc.compile()
    return nc


def prep_weights(Wg, Wu, Wd, IPAD):
    """Host-side re-tiling of the weights into the device DMA layouts."""
    Iin, Hh = Wg.shape
    HS = Hh // P
    IC = IPAD // ICW
    NO = Hh // OCW
    f32 = np.float32

    Wg_p = np.zeros((IPAD, Hh), f32)
    Wg_p[:Iin] = Wg
    Wu_p = np.zeros((IPAD, Hh), f32)
    Wu_p[:Iin] = Wu
    Wd_p = np.zeros((Hh, IPAD), f32)
    Wd_p[:, :Iin] = Wd

    # wg[ic, j, p, h2, ii] = Wg_p[ic*ICW + ii, (2j+h2)*128 + p]
    wg_host = np.ascontiguousarray(
        Wg_p.reshape(IC, ICW, HS // 2, 2, P).transpose(0, 2, 4, 3, 1)
    )
    wu_host = np.ascontiguousarray(
        Wu_p.reshape(IC, ICW, HS // 2, 2, P).transpose(0, 2, 4, 3, 1)
    )
    # wd[ic, osc, p, isub, oo] = Wd_p[osc*OCW + oo, ic*ICW + isub*128 + p]
    wd_host = np.ascontiguousarray(
        Wd_p.reshape(NO, OCW, IC, ICW // P, P).transpose(2, 0, 4, 3, 1)
    )
    return wg_host, wu_host, wd_host


def prep_x_shard(x2, c, T):
    """x2 [tokens, H] -> core c's [128, HS, T] tile layout."""
    Hh = x2.shape[1]
    xs = x2[c * T:(c + 1) * T]  # [T, H]
    return np.ascontiguousarray(xs.reshape(T, Hh // P, P).transpose(2, 1, 0))


def run_on_cores(nc, in_maps, **kwargs):
    return run_bass_kernel_spmd(nc, in_maps, core_ids=list(range(len(in_maps))), **kwargs)


_NC_CACHE = {}

# matmul dtype mode: "f32" (exact, 4 PE cycles/row) or "f32r" (tf32-like,
# 1 PE cycle/row, ~2e-4 rel err)
MM_MODE = "f32r"


def _get_nc(mode=None):
    mode = mode or MM_MODE
    key = (T, H, IPAD, mode)
    if key not in _NC_CACHE:
        _NC_CACHE[key] = build_nc(T, H, IPAD, mm_dt=(F32R if mode == "f32r" else F32))
    return _NC_CACHE[key]


def kernel(x, Wg, Wu, Wd, _trace=False, _trace_kwargs=None, _mode=None):
    x = np.asarray(x, np.float32)
    Wg = np.asarray(Wg, np.float32)
    Wu = np.asarray(Wu, np.float32)
    Wd = np.asarray(Wd, np.float32)

    nc = _get_nc(_mode)
    wg_host, wu_host, wd_host = prep_weights(Wg, Wu, Wd, IPAD)
    x2 = x.reshape(B * S, H)
    in_maps = [
        {
            "x": prep_x_shard(x2, c, T),
            "wg": wg_host,
            "wu": wu_host,
            "wd": wd_host,
        }
        for c in range(NCORES)
    ]
    kwargs = {}
    if _trace:
        kwargs["trace"] = True
        kwargs.update(_trace_kwargs or {})
    res = run_on_cores(nc, in_maps, **kwargs)
    shards = [res.results[c]["y"].reshape(T, H) for c in range(NCORES)]
    y = np.concatenate(shards, axis=0).reshape(B, S, H)
    if _trace:
        return y, res
    return y



# revision 2
# speedup vs baseline: 1.1485x; 1.1485x over previous
"""Fused SwiGLU MLP (gate/up/down) Trainium2 Bass kernel.

Problem: y = down( silu(x @ Wg^T) * (x @ Wu^T) ) with
  x  [B=2, S=2048, H=4096]  f32
  Wg [I=11008, H]           f32   (gate proj, [out,in])
  Wu [I=11008, H]           f32
  Wd [H, I]                 f32

Strategy: data-parallel over tokens across the 8 NeuronCores.
Each core gets T = 4096/8 = 512 tokens and the full (replicated) weights,
computing the entire MLP for its token shard.  No collectives; the host
just concatenates the 8 token shards.

Per-core work is 70.9 G MAC = 8256 matmuls of 128x128x512.  All matmul
operands are bf16 (1 PE cycle/row at 2.4 GHz, same rate as f32r, but half
the HBM traffic: ~270 MB/core vs 570 MB f32, so DMA stays well clear of
the PE roofline of ~1.76 ms).  f32 PSUM accumulation keeps the rel error
~4e-3.

Two-pass structure per core (v2; the v1 chunked structure interleaved
gate/up and down per i-chunk, costing PSUM-bank contention + a DVE
accumulate of y per chunk):
  pass 1: for each of 86 i-subtiles: psg/psu[128i, T] accumulate over
          32 h-subtiles; hm[:, isub, :] = silu(psg) * psu  (bf16, resident:
          full [128, 86, 512] = 86 KB/partition in SBUF)
  pass 2: for each o-chunk (8 x 512) hold 4 PSUM banks py[128t, 512o]
          (one per 128-token tile) and accumulate over all 86 i-subtiles;
          wd tile loaded once per (osc, isub), reused for 4 matmuls.
          Drain via ACT copy -> DMA out.  No DVE adds, no y memset.

All transposes/tiling are done on HOST in numpy so every device DMA is a
plain contiguous partition-major copy:
  x_host  [HS=32, 128, T]      x^T tiled: [hs, p, t] = x[t, hs*128+p]
  wg_host [NI=86, 128, HS, 128]  [isub, p, hs, ii] = Wg[isub*128+ii, hs*128+p]
  wu_host same
  wd_host [NO=8, NI, 128, 512]   [osc, isub, p, oo] = Wd[osc*512+oo, isub*128+p]
  y out   [TT=4, 128, H]       y[tt*128+p, o]  f32
"""

import numpy as np
import ml_dtypes

import concourse.bass as bass
import concourse.mybir as mybir
import concourse.tile as tile
from concourse import bacc
from concourse.bass_utils import run_bass_kernel_spmd

F32 = mybir.dt.float32
BF16 = mybir.dt.bfloat16
BF16_NP = ml_dtypes.bfloat16
P = 128
OCW = 512  # o-chunk width (one PSUM bank of f32)

# full-size problem constants
B, S, H, I = 2, 2048, 4096, 11008
NCORES = 8
T = (B * S) // NCORES  # 512 tokens per core


def build_nc(T, H, I, wg_bufs=3, wd_bufs=6, use_silu=True):
    assert T % P == 0 and T <= 512
    assert H % OCW == 0 and I % P == 0
    HS = H // P   # h subtiles (contraction for gate/up)
    NI = I // P   # i subtiles (with I padded to a multiple of 128 by host)
    NO = H // OCW
    TT = T // P

    nc = bacc.Bacc("TRN2", target_bir_lowering=False, debug=False)
    x_d = nc.dram_tensor("x", [HS, P, T], BF16, kind="ExternalInput").ap()
    wg_d = nc.dram_tensor("wg", [NI, P, HS, P], BF16, kind="ExternalInput").ap()
    wu_d = nc.dram_tensor("wu", [NI, P, HS, P], BF16, kind="ExternalInput").ap()
    wd_d = nc.dram_tensor("wd", [NO, NI, P, OCW], BF16, kind="ExternalInput").ap()
    y_d = nc.dram_tensor("y", [TT, P, H], F32, kind="ExternalOutput").ap()

    with tile.TileContext(nc) as tc:
        with (
            tc.tile_pool(name="xp", bufs=1) as xp,
            tc.tile_pool(name="hmp", bufs=1) as hmp,
            tc.tile_pool(name="wgp", bufs=wg_bufs) as wgp,
            tc.tile_pool(name="wup", bufs=wg_bufs) as wup,
            tc.tile_pool(name="wdp", bufs=wd_bufs) as wdp,
            tc.tile_pool(name="sgp", bufs=2) as sgp,
            tc.tile_pool(name="yop", bufs=4) as yop,
            tc.tile_pool(name="ps", bufs=8, space="PSUM") as ps,
        ):
            # resident x^T (4 MiB bf16), loaded per h-subtile so the first
            # matmuls don't wait on the whole tensor
            xt = xp.tile([P, HS, T], BF16)
            for hs in range(HS):
                nc.sync.dma_start(out=xt[:, hs, :], in_=x_d[hs])
            # resident h_mid, [128i, isub, t] bf16 (86 KB/partition)
            hm = hmp.tile([P, NI, T], BF16)

            # ---- pass 1: gate/up projections + silu*up ----
            for isub in range(NI):
                gt = wgp.tile([P, HS, P], BF16, tag="wg")
                nc.sync.dma_start(out=gt, in_=wg_d[isub])
                ut = wup.tile([P, HS, P], BF16, tag="wu")
                nc.sync.dma_start(out=ut, in_=wu_d[isub])
                psg = ps.tile([P, T], F32, tag="ps", name="psg")
                psu = ps.tile([P, T], F32, tag="ps", name="psu")
                for hs in range(HS):
                    first, last = hs == 0, hs == HS - 1
                    nc.tensor.matmul(psg, gt[:, hs, :], xt[:, hs, :],
                                     start=first, stop=last)
                    nc.tensor.matmul(psu, ut[:, hs, :], xt[:, hs, :],
                                     start=first, stop=last)
                sg = sgp.tile([P, T], F32, tag="sg")
                if use_silu:
                    # native HW silu: one ACT op frees psg immediately
                    nc.scalar.activation(sg, psg,
                                         mybir.ActivationFunctionType.Silu)
                else:
                    # CoreSim lacks Silu: sigmoid + extra DVE mul
                    nc.scalar.activation(sg, psg,
                                         mybir.ActivationFunctionType.Sigmoid)
                    nc.vector.tensor_mul(sg, sg, psg)
                nc.vector.tensor_mul(hm[:, isub, :], sg, psu)

            # ---- pass 2: down projection ----
            for osc in range(NO):
                pys = [ps.tile([P, OCW], F32, tag="ps", name=f"py{tt}")
                       for tt in range(TT)]
                for isub in range(NI):
                    wdt = wdp.tile([P, OCW], BF16, tag="wd")
                    nc.sync.dma_start(out=wdt, in_=wd_d[osc, isub])
                    for tt in range(TT):
                        nc.tensor.matmul(
                            pys[tt],
                            hm[:, isub, tt * P:(tt + 1) * P],
                            wdt,
                            start=(isub == 0), stop=(isub == NI - 1),
                        )
                for tt in range(TT):
                    yo = yop.tile([P, OCW], F32, tag="yo")
                    nc.scalar.activation(yo, pys[tt],
                                         mybir.ActivationFunctionType.Copy)
                    nc.sync.dma_start(
                        out=y_d[tt, :, osc * OCW:(osc + 1) * OCW], in_=yo)

    nc.compile()
    return nc


def prep_weights(Wg, Wu, Wd):
    """Host-side re-tiling of the weights into the device DMA layouts (bf16).

    Pads I up to a multiple of 128 with zeros (no-op for I=11008=86*128);
    padded hm columns are silu(0)*0 = 0 so they contribute nothing to y.
    """
    Iin, Hh = Wg.shape
    HS = Hh // P
    NI = -(-Iin // P)
    IPAD = NI * P
    NO = Hh // OCW

    if IPAD != Iin:
        Wg_p = np.zeros((IPAD, Hh), np.float32)
        Wg_p[:Iin] = Wg
        Wu_p = np.zeros((IPAD, Hh), np.float32)
        Wu_p[:Iin] = Wu
        Wd_p = np.zeros((Hh, IPAD), np.float32)
        Wd_p[:, :Iin] = Wd
    else:
        Wg_p, Wu_p, Wd_p = Wg, Wu, Wd

    # wg[isub, p, hs, ii] = Wg_p[isub*128 + ii, hs*128 + p]
    wg_host = np.ascontiguousarray(
        Wg_p.reshape(NI, P, HS, P).transpose(0, 3, 2, 1).astype(BF16_NP))
    wu_host = np.ascontiguousarray(
        Wu_p.reshape(NI, P, HS, P).transpose(0, 3, 2, 1).astype(BF16_NP))
    # wd[osc, isub, p, oo] = Wd_p[osc*512 + oo, isub*128 + p]
    wd_host = np.ascontiguousarray(
        Wd_p.reshape(NO, OCW, NI, P).transpose(0, 2, 3, 1).astype(BF16_NP))
    return wg_host, wu_host, wd_host


def prep_x_shard(x2, c, T):
    """x2 [tokens, H] -> core c's [HS, 128, T] bf16 tile layout."""
    Hh = x2.shape[1]
    xs = x2[c * T:(c + 1) * T]  # [T, H]
    return np.ascontiguousarray(
        xs.reshape(T, Hh // P, P).transpose(1, 2, 0).astype(BF16_NP))


def run_on_cores(nc, in_maps, **kwargs):
    return run_bass_kernel_spmd(nc, in_maps, core_ids=list(range(len(in_maps))), **kwargs)


_NC_CACHE = {}


def _get_nc():
    key = (T, H, I)
    if key not in _NC_CACHE:
        _NC_CACHE[key] = build_nc(T, H, I)
    return _NC_CACHE[key]


def kernel(x, Wg, Wu, Wd, _trace=False, _trace_kwargs=None):
    x = np.asarray(x, np.float32)
    Wg = np.asarray(Wg, np.float32)
    Wu = np.asarray(Wu, np.float32)
    Wd = np.asarray(Wd, np.float32)

    nc = _get_nc()
    wg_host, wu_host, wd_host = prep_weights(Wg, Wu, Wd)
    x2 = x.reshape(B * S, H)
    in_maps = [
        {
            "x": prep_x_shard(x2, c, T),
            "wg": wg_host,
            "wu": wu_host,
            "wd": wd_host,
        }
        for c in range(NCORES)
    ]
    kwargs = {}
    if _trace:
        kwargs["trace"] = True
        kwargs.update(_trace_kwargs or {})
    res = run_on_cores(nc, in_maps, **kwargs)
    shards = [res.results[c]["y"].reshape(T, H) for c in range(NCORES)]
    y = np.concatenate(shards, axis=0).reshape(B, S, H)
    if _trace:
        return y, res
    return y


# revision 4
# speedup vs baseline: 1.1545x; 1.0053x over previous
"""Fused SwiGLU MLP (gate/up/down) Trainium2 Bass kernel.

Problem: y = down( silu(x @ Wg^T) * (x @ Wu^T) ) with
  x  [B=2, S=2048, H=4096]  f32
  Wg [I=11008, H]           f32   (gate proj, [out,in])
  Wu [I=11008, H]           f32
  Wd [H, I]                 f32

Strategy: data-parallel over tokens across the 8 NeuronCores.
Each core gets T = 4096/8 = 512 tokens and the full (replicated) weights,
computing the entire MLP for its token shard.  No collectives; the host
just concatenates the 8 token shards.

Per-core work is 70.9 G MAC = 8256 matmuls of 128x128x512.  All matmul
operands are bf16 (1 PE cycle/row at 2.4 GHz, same rate as f32r, but half
the HBM traffic: ~270 MB/core vs 570 MB f32, so DMA stays well clear of
the PE roofline of ~1.76 ms).  f32 PSUM accumulation keeps the rel error
~4e-3.

Two-pass structure per core (v2; the v1 chunked structure interleaved
gate/up and down per i-chunk, costing PSUM-bank contention + a DVE
accumulate of y per chunk):
  pass 1: for each of 86 i-subtiles: psg/psu[128i, T] accumulate over
          32 h-subtiles; hm[:, isub, :] = silu(psg) * psu  (bf16, resident:
          full [128, 86, 512] = 86 KB/partition in SBUF)
  pass 2: for each o-chunk (8 x 512) hold 4 PSUM banks py[128t, 512o]
          (one per 128-token tile) and accumulate over all 86 i-subtiles;
          wd tile loaded once per (osc, isub), reused for 4 matmuls.
          Drain via ACT copy -> DMA out.  No DVE adds, no y memset.

All transposes/tiling are done on HOST in numpy so every device DMA is a
plain contiguous partition-major copy:
  x_host  [HS=32, 128, T]      x^T tiled: [hs, p, t] = x[t, hs*128+p]
  wg_host [NI=86, 128, HS, 128]  [isub, p, hs, ii] = Wg[isub*128+ii, hs*128+p]
  wu_host same
  wd_host [NO=8, NI, 128, 512]   [osc, isub, p, oo] = Wd[osc*512+oo, isub*128+p]
  y out   [TT=4, 128, H]       y[tt*128+p, o]  f32
"""

import numpy as np
import ml_dtypes

import concourse.bass as bass
import concourse.mybir as mybir
import concourse.tile as tile
from concourse import bacc
from concourse.bass_utils import run_bass_kernel_spmd

F32 = mybir.dt.float32
BF16 = mybir.dt.bfloat16
BF16_NP = ml_dtypes.bfloat16
P = 128
OCW = 512  # o-chunk width (one PSUM bank of f32)

# full-size problem constants
B, S, H, I = 2, 2048, 4096, 11008
NCORES = 8
T = (B * S) // NCORES  # 512 tokens per core


def build_nc(T, H, I, wg_bufs=3, wd_bufs=6, use_silu=True):
    assert T % P == 0 and T <= 512
    assert H % OCW == 0 and I % P == 0
    HS = H // P   # h subtiles (contraction for gate/up)
    NI = I // P   # i subtiles (with I padded to a multiple of 128 by host)
    NO = H // OCW
    TT = T // P

    nc = bacc.Bacc("TRN2", target_bir_lowering=False, debug=False)
    x_d = nc.dram_tensor("x", [HS, P, T], BF16, kind="ExternalInput").ap()
    wg_d = nc.dram_tensor("wg", [NI, P, HS, P], BF16, kind="ExternalInput").ap()
    wu_d = nc.dram_tensor("wu", [NI, P, HS, P], BF16, kind="ExternalInput").ap()
    wd_d = nc.dram_tensor("wd", [NO, NI, P, OCW], BF16, kind="ExternalInput").ap()
    y_d = nc.dram_tensor("y", [TT, P, H], F32, kind="ExternalOutput").ap()

    with tile.TileContext(nc) as tc:
        with (
            tc.tile_pool(name="xp", bufs=1) as xp,
            tc.tile_pool(name="hmp", bufs=1) as hmp,
            tc.tile_pool(name="wgp", bufs=wg_bufs) as wgp,
            tc.tile_pool(name="wup", bufs=wg_bufs) as wup,
            tc.tile_pool(name="wdp", bufs=wd_bufs) as wdp,
            tc.tile_pool(name="sgp", bufs=2) as sgp,
            tc.tile_pool(name="yop", bufs=4) as yop,
            tc.tile_pool(name="ps", bufs=8, space="PSUM") as ps,
        ):
            # DMA issue order matters: queues are FIFO, so put the first
            # i-subtile's weights ahead of the bulk x load or the first
            # matmul waits behind 4 MiB of x.
            gt0 = wgp.tile([P, HS, P], BF16, tag="wg")
            nc.sync.dma_start(out=gt0, in_=wg_d[0])
            xt = xp.tile([P, HS, T], BF16)
            nc.sync.dma_start(out=xt[:, 0, :], in_=x_d[0])
            ut0 = wup.tile([P, HS, P], BF16, tag="wu")
            nc.sync.dma_start(out=ut0, in_=wu_d[0])
            for hs in range(1, HS):
                nc.sync.dma_start(out=xt[:, hs, :], in_=x_d[hs])
            # resident h_mid, [128i, isub, t] bf16 (86 KB/partition)
            hm = hmp.tile([P, NI, T], BF16)

            # ---- pass 1: gate/up projections + silu*up ----
            for isub in range(NI):
                if isub == 0:
                    gt, ut = gt0, ut0
                else:
                    gt = wgp.tile([P, HS, P], BF16, tag="wg")
                    nc.sync.dma_start(out=gt, in_=wg_d[isub])
                    ut = wup.tile([P, HS, P], BF16, tag="wu")
                    nc.sync.dma_start(out=ut, in_=wu_d[isub])
                psg = ps.tile([P, T], F32, tag="ps", name="psg")
                psu = ps.tile([P, T], F32, tag="ps", name="psu")
                for hs in range(HS):
                    first, last = hs == 0, hs == HS - 1
                    nc.tensor.matmul(psg, gt[:, hs, :], xt[:, hs, :],
                                     start=first, stop=last)
                    nc.tensor.matmul(psu, ut[:, hs, :], xt[:, hs, :],
                                     start=first, stop=last)
                sg = sgp.tile([P, T], F32, tag="sg")
                if use_silu:
                    # native HW silu: one ACT op frees psg immediately
                    nc.scalar.activation(sg, psg,
                                         mybir.ActivationFunctionType.Silu)
                else:
                    # CoreSim lacks Silu: sigmoid + extra DVE mul
                    nc.scalar.activation(sg, psg,
                                         mybir.ActivationFunctionType.Sigmoid)
                    nc.vector.tensor_mul(sg, sg, psg)
                nc.vector.tensor_mul(hm[:, isub, :], sg, psu)

            # ---- pass 2: down projection ----
            # The drain of o-chunk osc (PSUM->SBUF copy + y DMA) is issued
            # only after the first DRAIN_AT wd loads of chunk osc+1: the y
            # DMAs block (waiting on the copies) at the head of the FIFO DMA
            # queues, and anything issued behind them stalls, starving the PE
            # at every chunk boundary.
            DRAIN_AT = 8

            def drain(osc, pys):
                for tt in range(TT):
                    yo = yop.tile([P, OCW], F32, tag="yo")
                    nc.scalar.activation(yo, pys[tt],
                                         mybir.ActivationFunctionType.Copy)
                    nc.sync.dma_start(
                        out=y_d[tt, :, osc * OCW:(osc + 1) * OCW], in_=yo)

            pend = None
            for osc in range(NO):
                pys = [ps.tile([P, OCW], F32, tag="ps", name=f"py{tt}")
                       for tt in range(TT)]
                for isub in range(NI):
                    wdt = wdp.tile([P, OCW], BF16, tag="wd")
                    nc.sync.dma_start(out=wdt, in_=wd_d[osc, isub])
                    for tt in range(TT):
                        nc.tensor.matmul(
                            pys[tt],
                            hm[:, isub, tt * P:(tt + 1) * P],
                            wdt,
                            start=(isub == 0), stop=(isub == NI - 1),
                        )
                    if isub == DRAIN_AT and pend is not None:
                        drain(*pend)
                        pend = None
                pend = (osc, pys)
            drain(*pend)

    nc.compile()
    return nc


def prep_weights(Wg, Wu, Wd):
    """Host-side re-tiling of the weights into the device DMA layouts (bf16).

    Pads I up to a multiple of 128 with zeros (no-op for I=11008=86*128);
    padded hm columns are silu(0)*0 = 0 so they contribute nothing to y.
    """
    Iin, Hh = Wg.shape
    HS = Hh // P
    NI = -(-Iin // P)
    IPAD = NI * P
    NO = Hh // OCW

    if IPAD != Iin:
        Wg_p = np.zeros((IPAD, Hh), np.float32)
        Wg_p[:Iin] = Wg
        Wu_p = np.zeros((IPAD, Hh), np.float32)
        Wu_p[:Iin] = Wu
        Wd_p = np.zeros((Hh, IPAD), np.float32)
        Wd_p[:, :Iin] = Wd
    else:
        Wg_p, Wu_p, Wd_p = Wg, Wu, Wd

    # wg[isub, p, hs, ii] = Wg_p[isub*128 + ii, hs*128 + p]
    wg_host = np.ascontiguousarray(
        Wg_p.reshape(NI, P, HS, P).transpose(0, 3, 2, 1).astype(BF16_NP))
    wu_host = np.ascontiguousarray(
        Wu_p.reshape(NI, P, HS, P).transpose(0, 3, 2, 1).astype(BF16_NP))
    # wd[osc, isub, p, oo] = Wd_p[osc*512 + oo, isub*128 + p]
    wd_host = np.ascontiguousarray(
        Wd_p.reshape(NO, OCW, NI, P).transpose(0, 2, 3, 1).astype(BF16_NP))
    return wg_host, wu_host, wd_host


def prep_x_shard(x2, c, T):
    """x2 [tokens, H] -> core c's [HS, 128, T] bf16 tile layout."""
    Hh = x2.shape[1]
    xs = x2[c * T:(c + 1) * T]  # [T, H]
    return np.ascontiguousarray(
        xs.reshape(T, Hh // P, P).transpose(1, 2, 0).astype(BF16_NP))


def run_on_cores(nc, in_maps, **kwargs):
    return run_bass_kernel_spmd(nc, in_maps, core_ids=list(range(len(in_maps))), **kwargs)


_NC_CACHE = {}


def _get_nc():
    key = (T, H, I)
    if key not in _NC_CACHE:
        _NC_CACHE[key] = build_nc(T, H, I)
    return _NC_CACHE[key]


def kernel(x, Wg, Wu, Wd, _trace=False, _trace_kwargs=None):
    x = np.asarray(x, np.float32)
    Wg = np.asarray(Wg, np.float32)
    Wu = np.asarray(Wu, np.float32)
    Wd = np.asarray(Wd, np.float32)

    nc = _get_nc()
    wg_host, wu_host, wd_host = prep_weights(Wg, Wu, Wd)
    x2 = x.reshape(B * S, H)
    in_maps = [
        {
            "x": prep_x_shard(x2, c, T),
            "wg": wg_host,
            "wu": wu_host,
            "wd": wd_host,
        }
        for c in range(NCORES)
    ]
    kwargs = {}
    if _trace:
        kwargs["trace"] = True
        kwargs.update(_trace_kwargs or {})
    res = run_on_cores(nc, in_maps, **kwargs)
    shards = [res.results[c]["y"].reshape(T, H) for c in range(NCORES)]
    y = np.concatenate(shards, axis=0).reshape(B, S, H)
    if _trace:
        return y, res
    return y


# revision 9
# speedup vs baseline: 1.1679x; 1.0116x over previous
"""Fused SwiGLU MLP (gate/up/down) Trainium2 Bass kernel.

Problem: y = down( silu(x @ Wg^T) * (x @ Wu^T) ) with
  x  [B=2, S=2048, H=4096]  f32
  Wg [I=11008, H]           f32   (gate proj, [out,in])
  Wu [I=11008, H]           f32
  Wd [H, I]                 f32

Strategy: data-parallel over tokens across the 8 NeuronCores.
Each core gets T = 4096/8 = 512 tokens and the full (replicated) weights,
computing the entire MLP for its token shard.  No collectives; the host
just concatenates the 8 token shards.

Per-core work is 70.9 G MAC = 8256 matmuls of 128x128x512.  All matmul
operands are bf16 (1 PE cycle/row at 2.4 GHz, same rate as f32r, but half
the HBM traffic: ~270 MB/core vs 570 MB f32, so DMA stays well clear of
the PE roofline of ~1.76 ms).  f32 PSUM accumulation keeps the rel error
~4e-3.

Two-pass structure per core (v2; the v1 chunked structure interleaved
gate/up and down per i-chunk, costing PSUM-bank contention + a DVE
accumulate of y per chunk):
  pass 1: for each of 86 i-subtiles: psg/psu[128i, T] accumulate over
          32 h-subtiles; hm[:, isub, :] = silu(psg) * psu  (bf16, resident:
          full [128, 86, 512] = 86 KB/partition in SBUF)
  pass 2: for each o-chunk (8 x 512) hold 4 PSUM banks py[128t, 512o]
          (one per 128-token tile) and accumulate over all 86 i-subtiles;
          wd tile loaded once per (osc, isub), reused for 4 matmuls.
          Drain via ACT copy -> DMA out.  No DVE adds, no y memset.

All transposes/tiling are done on HOST in numpy so every device DMA is a
plain contiguous partition-major copy:
  x_host  [HS=32, 128, T]      x^T tiled: [hs, p, t] = x[t, hs*128+p]
  wg_host [NI=86, 128, HS, 128]  [isub, p, hs, ii] = Wg[isub*128+ii, hs*128+p]
  wu_host same
  wd_host [NO=8, NI, 128, 512]   [osc, isub, p, oo] = Wd[osc*512+oo, isub*128+p]
  y out   [TT=4, 128, H]       y[tt*128+p, o]  f32
"""

import numpy as np
import ml_dtypes

import concourse.bass as bass
import concourse.mybir as mybir
import concourse.tile as tile
from concourse import bacc
from concourse.bass_utils import run_bass_kernel_spmd

F32 = mybir.dt.float32
BF16 = mybir.dt.bfloat16
BF16_NP = ml_dtypes.bfloat16
P = 128
OCW = 512  # o-chunk width (one PSUM bank of f32)

# full-size problem constants
B, S, H, I = 2, 2048, 4096, 11008
NCORES = 8
T = (B * S) // NCORES  # 512 tokens per core


def build_nc(T, H, I, wg_bufs=3, wd_bufs=16, use_silu=True):
    assert T % P == 0 and T <= 512
    assert H % OCW == 0 and I % P == 0
    HS = H // P   # h subtiles (contraction for gate/up)
    NI = I // P   # i subtiles (with I padded to a multiple of 128 by host)
    NO = H // OCW
    TT = T // P

    nc = bacc.Bacc("TRN2", target_bir_lowering=False, debug=False)
    x_d = nc.dram_tensor("x", [HS, P, T], BF16, kind="ExternalInput").ap()
    wg_d = nc.dram_tensor("wg", [NI, P, HS, P], BF16, kind="ExternalInput").ap()
    wu_d = nc.dram_tensor("wu", [NI, P, HS, P], BF16, kind="ExternalInput").ap()
    wd_d = nc.dram_tensor("wd", [NO, NI, P, OCW], BF16, kind="ExternalInput").ap()
    y_d = nc.dram_tensor("y", [TT, P, H], F32, kind="ExternalOutput").ap()

    with tile.TileContext(nc) as tc:
        with (
            tc.tile_pool(name="xp", bufs=1) as xp,
            tc.tile_pool(name="hmp", bufs=1) as hmp,
            tc.tile_pool(name="wgp", bufs=wg_bufs) as wgp,
            tc.tile_pool(name="wup", bufs=wg_bufs) as wup,
            tc.tile_pool(name="wdp", bufs=wd_bufs) as wdp,
            tc.tile_pool(name="sgp", bufs=2) as sgp,
            tc.tile_pool(name="yop", bufs=4) as yop,
            tc.tile_pool(name="ps", bufs=8, space="PSUM") as ps,
        ):
            # DMA issue order matters: queues are FIFO, so put the first two
            # i-subtiles' weights ahead of the bulk x load or the first
            # matmuls wait behind 4 MiB of x.
            head = []
            xt = None
            for isub in range(2):
                gt = wgp.tile([P, HS, P], BF16, tag="wg", name=f"gt{isub}")
                nc.sync.dma_start(out=gt, in_=wg_d[isub])
                if isub == 0:
                    xt = xp.tile([P, HS, T], BF16, name="xt")
                    nc.sync.dma_start(out=xt[:, 0, :], in_=x_d[0])
                ut = wup.tile([P, HS, P], BF16, tag="wu", name=f"ut{isub}")
                nc.sync.dma_start(out=ut, in_=wu_d[isub])
                head.append((gt, ut))
            for hs in range(1, HS):
                nc.sync.dma_start(out=xt[:, hs, :], in_=x_d[hs])
            # resident h_mid, [128i, isub, t] bf16 (86 KB/partition)
            hm = hmp.tile([P, NI, T], BF16)

            # ---- pass 1: gate/up projections + silu*up ----
            for isub in range(NI):
                if isub < 2:
                    gt, ut = head[isub]
                else:
                    gt = wgp.tile([P, HS, P], BF16, tag="wg")
                    nc.sync.dma_start(out=gt, in_=wg_d[isub])
                    ut = wup.tile([P, HS, P], BF16, tag="wu")
                    nc.sync.dma_start(out=ut, in_=wu_d[isub])
                psg = ps.tile([P, T], F32, tag="ps", name="psg")
                psu = ps.tile([P, T], F32, tag="ps", name="psu")
                for hs in range(HS):
                    first, last = hs == 0, hs == HS - 1
                    nc.tensor.matmul(psg, gt[:, hs, :], xt[:, hs, :],
                                     start=first, stop=last)
                    nc.tensor.matmul(psu, ut[:, hs, :], xt[:, hs, :],
                                     start=first, stop=last)
                sg = sgp.tile([P, T], F32, tag="sg")
                if use_silu:
                    # native HW silu: one ACT op frees psg immediately
                    nc.scalar.activation(sg, psg,
                                         mybir.ActivationFunctionType.Silu)
                else:
                    # CoreSim lacks Silu: sigmoid + extra DVE mul
                    nc.scalar.activation(sg, psg,
                                         mybir.ActivationFunctionType.Sigmoid)
                    nc.vector.tensor_mul(sg, sg, psg)
                nc.vector.tensor_mul(hm[:, isub, :], sg, psu)

            # ---- pass 2: down projection ----
            # The drain of o-chunk osc (PSUM->SBUF copy + y DMA) is issued
            # only after the first DRAIN_AT wd loads of chunk osc+1: the y
            # DMAs block (waiting on the copies) at the head of the FIFO DMA
            # queues, and anything issued behind them stalls, starving the PE
            # at every chunk boundary.
            DRAIN_AT = 24

            def drain(osc, pys):
                # alternate ACT/DVE so the four copies run pairwise-parallel
                for tt in range(TT):
                    yo = yop.tile([P, OCW], F32, tag="yo")
                    if tt % 2 == 0:
                        nc.scalar.activation(yo, pys[tt],
                                             mybir.ActivationFunctionType.Copy)
                    else:
                        nc.vector.tensor_scalar_mul(yo, pys[tt], 1.0)
                    nc.sync.dma_start(
                        out=y_d[tt, :, osc * OCW:(osc + 1) * OCW], in_=yo)

            pend = None
            for osc in range(NO):
                pys = [ps.tile([P, OCW], F32, tag="ps", name=f"py{tt}")
                       for tt in range(TT)]
                for isub in range(NI):
                    wdt = wdp.tile([P, OCW], BF16, tag="wd")
                    nc.sync.dma_start(out=wdt, in_=wd_d[osc, isub])
                    for tt in range(TT):
                        nc.tensor.matmul(
                            pys[tt],
                            hm[:, isub, tt * P:(tt + 1) * P],
                            wdt,
                            start=(isub == 0), stop=(isub == NI - 1),
                        )
                    if isub == DRAIN_AT and pend is not None:
                        drain(*pend)
                        pend = None
                pend = (osc, pys)
            drain(*pend)

    nc.compile()
    return nc


def prep_weights(Wg, Wu, Wd):
    """Host-side re-tiling of the weights into the device DMA layouts (bf16).

    Pads I up to a multiple of 128 with zeros (no-op for I=11008=86*128);
    padded hm columns are silu(0)*0 = 0 so they contribute nothing to y.
    """
    Iin, Hh = Wg.shape
    HS = Hh // P
    NI = -(-Iin // P)
    IPAD = NI * P
    NO = Hh // OCW

    if IPAD != Iin:
        Wg_p = np.zeros((IPAD, Hh), np.float32)
        Wg_p[:Iin] = Wg
        Wu_p = np.zeros((IPAD, Hh), np.float32)
        Wu_p[:Iin] = Wu
        Wd_p = np.zeros((Hh, IPAD), np.float32)
        Wd_p[:, :Iin] = Wd
    else:
        Wg_p, Wu_p, Wd_p = Wg, Wu, Wd

    # wg[isub, p, hs, ii] = Wg_p[isub*128 + ii, hs*128 + p]
    wg_host = np.ascontiguousarray(
        Wg_p.reshape(NI, P, HS, P).transpose(0, 3, 2, 1).astype(BF16_NP))
    wu_host = np.ascontiguousarray(
        Wu_p.reshape(NI, P, HS, P).transpose(0, 3, 2, 1).astype(BF16_NP))
    # wd[osc, isub, p, oo] = Wd_p[osc*512 + oo, isub*128 + p]
    wd_host = np.ascontiguousarray(
        Wd_p.reshape(NO, OCW, NI, P).transpose(0, 2, 3, 1).astype(BF16_NP))
    return wg_host, wu_host, wd_host


def prep_x_shard(x2, c, T):
    """x2 [tokens, H] -> core c's [HS, 128, T] bf16 tile layout."""
    Hh = x2.shape[1]
    xs = x2[c * T:(c + 1) * T]  # [T, H]
    return np.ascontiguousarray(
        xs.reshape(T, Hh // P, P).transpose(1, 2, 0).astype(BF16_NP))


def run_on_cores(nc, in_maps, **kwargs):
    return run_bass_kernel_spmd(nc, in_maps, core_ids=list(range(len(in_maps))), **kwargs)


_NC_CACHE = {}


def _get_nc():
    key = (T, H, I)
    if key not in _NC_CACHE:
        _NC_CACHE[key] = build_nc(T, H, I)
    return _NC_CACHE[key]


def kernel(x, Wg, Wu, Wd, _trace=False, _trace_kwargs=None):
    x = np.asarray(x, np.float32)
    Wg = np.asarray(Wg, np.float32)
    Wu = np.asarray(Wu, np.float32)
    Wd = np.asarray(Wd, np.float32)

    nc = _get_nc()
    wg_host, wu_host, wd_host = prep_weights(Wg, Wu, Wd)
    x2 = x.reshape(B * S, H)
    in_maps = [
        {
            "x": prep_x_shard(x2, c, T),
            "wg": wg_host,
            "wu": wu_host,
            "wd": wd_host,
        }
        for c in range(NCORES)
    ]
    kwargs = {}
    if _trace:
        kwargs["trace"] = True
        kwargs.update(_trace_kwargs or {})
    res = run_on_cores(nc, in_maps, **kwargs)
    shards = [res.results[c]["y"].reshape(T, H) for c in range(NCORES)]
    y = np.concatenate(shards, axis=0).reshape(B, S, H)
    if _trace:
        return y, res
    return y
